# revision 7
# baseline (speedup 1.0000x reference)
"""HardTripletLoss (non-hardest branch) on 8 TRN2 NeuronCores — v2.2.

Math:  loss = mean_{i!=j} relu(d_pos[i] - pdist[i,j] + margin)
  pdist[i,j] = ||x_i||^2 + ||y_j||^2 - 2 x_i.y_j ,  d_pos = diag(pdist)
  =>  term(i,j) = relu(G[i,j] + a[i] - b[j])  with  G = 2 x y^T,
      a[i] = ||y_i||^2 - 2 x_i.y_i + margin,  b[j] = bf16(||y_j||^2).
Diagonal evaluates to ~relu(margin) = margin; host subtracts N*margin.

Host-side prep (O(N*D), ~0.01% of FLOPs): transpose + bf16-cast the matmul
operands, compute a and b.  Device does the O(N^2*D) matmul and the O(N^2)
relu/reduce epilogue.

Sharding: x rows split across 8 cores (data parallel), y replicated.
Per core: bf16 G tiles [128, UNIT_W] into PSUM; epilogue split between
 - DVE:  sum_j max(G+a, b) via fused scalar_tensor_tensor w/ accumulate
         (relu(z-b) = max(z,b) - b; host subtracts the known sum of b),
 - ACT:  PE folds -b into PSUM via K=1 matmuls against the b row vector,
         then activation(Relu, bias=a) with free-dim accumulate.
ACT units are emitted before DVE units inside each column block so the
epilogue can start before the (large) bb broadcast DMA lands.
Row-partial sums land in res [128, NU] per core; host reduces in f64.
"""

import sys

if "/opt/trn_rl_repo" not in sys.path:
    sys.path.insert(0, "/opt/trn_rl_repo")

import numpy as np

N, D = 8192, 128
NCORES = 8
SH = N // NCORES          # 1024 x-rows per core
MT = SH // 128            # 8 m-tiles
MARGIN = 0.2

UNIT_W = 2048             # epilogue tile width (psum: UNIT_W/512 banks)
NH = N // UNIT_W          # column blocks per row of m-tiles
NU = MT * NH              # total units per core
PS_BUFS = 8 * 512 // UNIT_W  # use all 8 psum banks


def _is_act(m, nh):
    # engine assignment: ACT (fold path) vs DVE (max-trick)
    return (m + nh) % 2 == 0


_cache = {}


def _build():
    import concourse.mybir as mybir
    from concourse import bacc
    from concourse.tile import TileContext

    f32 = mybir.dt.float32
    bf16 = mybir.dt.bfloat16
    Alu = mybir.AluOpType
    Act = mybir.ActivationFunctionType

    nc = bacc.Bacc()
    # host-pre-transposed operands: xt = (2x)^T [d, i], yt = y^T [d, j]
    xt = nc.declare_dram_parameter("xt", [128, SH], bf16, isOutput=False)
    yt = nc.declare_dram_parameter("yt", [128, N], bf16, isOutput=False)
    bv = nc.declare_dram_parameter("bv", [1, N], bf16, isOutput=False)    # b row
    av = nc.declare_dram_parameter("av", [128, MT], f32, isOutput=False)  # a cols
    out_res = nc.declare_dram_parameter("res", [128, NU], f32, isOutput=True)

    with TileContext(nc) as tc:
        with (
            tc.tile_pool(name="big", bufs=1) as big,
            tc.tile_pool(name="epd", bufs=3) as epd,
            tc.tile_pool(name="epa", bufs=3) as epa,
            tc.tile_pool(name="ps", bufs=PS_BUFS, space="PSUM") as ps,
        ):
            yT = big.tile([128, N], bf16)            # y^T  [d, j]
            xT = big.tile([128, SH], bf16)           # (2x)^T [d, i]
            bb = big.tile([128, N], bf16)            # b broadcast to all partitions
            brow = big.tile([1, N], bf16)            # b as a single row
            acol = big.tile([128, MT], f32)          # a per m-tile column
            negones = big.tile([1, 128], bf16)
            res = big.tile([128, NU], f32)

            nc.vector.memset(negones[:], -1.0)

            # ---- loads: yt chunks on sync ring; the rest on scalar ring ----
            for q in range(4):
                nc.sync.dma_start(
                    yT[:, q * 2048 : (q + 1) * 2048],
                    yt[:, q * 2048 : (q + 1) * 2048],
                )
            nc.scalar.dma_start(xT[:], xt[:, :])
            nc.scalar.dma_start(acol[:], av[:, :])
            nc.scalar.dma_start(brow[:], bv[:, :])
            nc.scalar.dma_start(bb[:], bv[0:1, :].broadcast_to([128, N]))

            # ---- main: G tiles + fused epilogue ----
            HW = UNIT_W // 512  # matmuls per unit
            for nh in range(NH):
                # ACT units first: they don't need the big bb broadcast
                ms = [m for m in range(MT) if _is_act(m, nh)] + [
                    m for m in range(MT) if not _is_act(m, nh)
                ]
                for m in ms:
                    col = m * NH + nh  # res column (m-major for host)
                    is_act = _is_act(m, nh)
                    pt = ps.tile([128, UNIT_W], f32, tag="g")
                    for h in range(HW):
                        nc.tensor.matmul(
                            pt[:, h * 512 : (h + 1) * 512],
                            lhsT=xT[:, m * 128 : (m + 1) * 128],
                            rhs=yT[:, nh * UNIT_W + h * 512 : nh * UNIT_W + (h + 1) * 512],
                            start=True, stop=not is_act,
                        )
                    if is_act:
                        for h in range(HW):
                            nc.tensor.matmul(
                                pt[:, h * 512 : (h + 1) * 512],
                                lhsT=negones[:],
                                rhs=brow[0:1, nh * UNIT_W + h * 512 : nh * UNIT_W + (h + 1) * 512],
                                start=False, stop=True,
                            )
                        scr = epa.tile([128, UNIT_W], bf16, tag="ep_act")
                        nc.scalar.activation(
                            scr[:], pt[:], Act.Relu,
                            bias=acol[:, m : m + 1],
                            accum_out=res[:, col : col + 1],
                        )
                    else:
                        scr = epd.tile([128, UNIT_W], bf16, tag="ep_dve")
                        nc.vector.scalar_tensor_tensor(
                            out=scr[:], in0=pt[:], scalar=acol[:, m : m + 1],
                            in1=bb[:, nh * UNIT_W : (nh + 1) * UNIT_W],
                            op0=Alu.add, op1=Alu.max,
                            accum_out=res[:, col : col + 1],
                        )

            nc.scalar.dma_start(out_res[:], res[:])

    return nc


def kernel(x: np.ndarray, y: np.ndarray) -> np.ndarray:
    from concourse.bass_utils import run_bass_kernel_spmd
    import ml_dtypes

    x = np.ascontiguousarray(x, dtype=np.float32)
    y = np.ascontiguousarray(y, dtype=np.float32)

    if "nc" not in _cache:
        nc = _build()
        if not nc.is_finalized():
            nc.finalize()
        _cache["nc"] = nc
    nc = _cache["nc"]

    # host-side O(N*D) prologue
    yy = np.sum(y.astype(np.float64) * y.astype(np.float64), axis=1)
    b16 = yy.astype(np.float32).astype(ml_dtypes.bfloat16)
    z2 = 2.0 * np.sum(x.astype(np.float64) * y.astype(np.float64), axis=1)
    a = (yy - z2 + MARGIN).astype(np.float32)

    ytT = np.ascontiguousarray(y.T).astype(ml_dtypes.bfloat16)        # [128, N]
    xtT = np.ascontiguousarray((2.0 * x).T).astype(ml_dtypes.bfloat16)  # [128, N]
    bv = b16.reshape(1, N)
    in_maps = []
    for c in range(NCORES):
        sl = slice(c * SH, (c + 1) * SH)
        in_maps.append({
            "xt": np.ascontiguousarray(xtT[:, sl]),
            "yt": ytT,
            "bv": bv,
            "av": np.ascontiguousarray(a[sl].reshape(MT, 128).T),  # [128, MT]
        })

    _cache["in_maps"] = in_maps
    out = run_bass_kernel_spmd(nc, in_maps, list(range(NCORES)))
    results = out.results

    # host reduction (f64)
    total = 0.0
    for c in range(NCORES):
        total += np.asarray(results[c]["res"], dtype=np.float64).sum()
    bsum_blk = b16.astype(np.float64).reshape(NH, UNIT_W).sum(axis=1)
    # subtract Sum_b for every DVE (max-trick) unit
    for nh in range(NH):
        n_dve = sum(1 for m in range(MT) if not _is_act(m, nh))
        total -= NCORES * n_dve * 128.0 * bsum_blk[nh]
    total -= float(N) * float(np.float32(MARGIN))
    return np.float32(total / (float(N) * float(N)))


# revision 9
# speedup vs baseline: 1.0674x; 1.0674x over previous
"""HardTripletLoss (non-hardest branch) on 8 TRN2 NeuronCores — v2.3.

Math:  loss = mean_{i!=j} relu(d_pos[i] - pdist[i,j] + margin)
  pdist[i,j] = ||x_i||^2 + ||y_j||^2 - 2 x_i.y_j ,  d_pos = diag(pdist)
  =>  term(i,j) = relu(G[i,j] + a[i] - b[j])  with  G = 2 x y^T,
      a[i] = ||y_i||^2 - 2 x_i.y_i + margin,  b[j] = bf16(||y_j||^2).
Diagonal evaluates to ~relu(margin) = margin; host subtracts N*margin.

Host-side prep (O(N*D)): transpose + fp8/bf16-cast the matmul operands,
compute a and b.  Device does the O(N^2*D) matmul and the O(N^2)
relu/reduce epilogue.

G matmuls run in fp8 e4m3 DoubleRow mode (2 K-subtiles of 64 on 64
partitions -> 0.5 cycles/row).  Epilogue alternates DVE (max-trick) and
ACT (PE folds -b via K=1 bf16 matmuls, then Relu w/ bias=a) per m-tile.
Row-partial sums land in res [128, NU] per core; host reduces in f64.
"""

import sys

if "/opt/trn_rl_repo" not in sys.path:
    sys.path.insert(0, "/opt/trn_rl_repo")

import numpy as np

N, D = 8192, 128
NCORES = 8
SH = N // NCORES          # 1024 x-rows per core
MT = SH // 128            # 8 m-tiles
MARGIN = 0.2

UNIT_W = 1024             # epilogue tile width (psum: UNIT_W/512 banks)
NH = N // UNIT_W          # column blocks per row of m-tiles
NU = MT * NH              # total units per core
PS_BUFS = 8 * 512 // UNIT_W  # use all 8 psum banks
USE_FP8 = True


def _is_act(m, nh):
    # engine assignment: ACT (fold path) vs DVE (max-trick), alternating
    return (m + nh) % 2 == 0


_cache = {}


def _build():
    import concourse.mybir as mybir
    from concourse import bacc
    from concourse.tile import TileContext

    f32 = mybir.dt.float32
    bf16 = mybir.dt.bfloat16
    fp8 = mybir.dt.float8e4
    mmdt = fp8 if USE_FP8 else bf16
    Alu = mybir.AluOpType
    Act = mybir.ActivationFunctionType
    DR = mybir.MatmulPerfMode.DoubleRow

    nc = bacc.Bacc()
    # host-pre-transposed operands: xt = (2x)^T, yt = y^T
    # fp8 DoubleRow layout: [64, 2, cols], contraction d = k2*64 + p
    if USE_FP8:
        xt = nc.declare_dram_parameter("xt", [64, 2 * SH], mmdt, isOutput=False)
        yt = nc.declare_dram_parameter("yt", [64, 2 * N], mmdt, isOutput=False)
    else:
        xt = nc.declare_dram_parameter("xt", [128, SH], mmdt, isOutput=False)
        yt = nc.declare_dram_parameter("yt", [128, N], mmdt, isOutput=False)
    bv = nc.declare_dram_parameter("bv", [1, N], bf16, isOutput=False)    # b row
    av = nc.declare_dram_parameter("av", [128, MT], f32, isOutput=False)  # a cols
    out_res = nc.declare_dram_parameter("res", [128, NU], f32, isOutput=True)

    with TileContext(nc) as tc:
        with (
            tc.tile_pool(name="big", bufs=1) as big,
            tc.tile_pool(name="epd", bufs=3) as epd,
            tc.tile_pool(name="epa", bufs=3) as epa,
            tc.tile_pool(name="ps", bufs=PS_BUFS, space="PSUM") as ps,
        ):
            if USE_FP8:
                yT = big.tile([64, 2, N], mmdt)      # y^T  [p, k2, j]
                xT = big.tile([64, 2, SH], mmdt)     # (2x)^T [p, k2, i]
            else:
                yT = big.tile([128, N], mmdt)
                xT = big.tile([128, SH], mmdt)
            bb = big.tile([128, N], bf16)            # b broadcast to all partitions
            brow = big.tile([1, N], bf16)            # b as a single row
            acol = big.tile([128, MT], f32)          # a per m-tile column
            negones = big.tile([1, 128], bf16)
            res = big.tile([128, NU], f32)

            nc.vector.memset(negones[:], -1.0)

            # ---- loads: yt chunks on sync ring; the rest on scalar ring ----
            for q in range(4):
                if USE_FP8:
                    nc.sync.dma_start(
                        yT[:, :, q * 2048 : (q + 1) * 2048],
                        yt[:, :].rearrange("p (k c) -> p k c", k=2)[
                            :, :, q * 2048 : (q + 1) * 2048
                        ],
                    )
                else:
                    nc.sync.dma_start(
                        yT[:, q * 2048 : (q + 1) * 2048],
                        yt[:, q * 2048 : (q + 1) * 2048],
                    )
            if USE_FP8:
                nc.scalar.dma_start(
                    xT[:], xt[:, :].rearrange("p (k c) -> p k c", k=2)
                )
            else:
                nc.scalar.dma_start(xT[:], xt[:, :])
            nc.scalar.dma_start(bb[:], bv[0:1, :].broadcast_to([128, N]))
            nc.scalar.dma_start(acol[:], av[:, :])
            nc.scalar.dma_start(brow[:], bv[:, :])

            # ---- main: G tiles + fused epilogue ----
            HW = UNIT_W // 512  # matmuls per unit
            for nh in range(NH):
                for m in range(MT):
                    col = m * NH + nh  # res column (m-major for host)
                    is_act = _is_act(m, nh)
                    pt = ps.tile([128, UNIT_W], f32, tag="g")
                    for h in range(HW):
                        c0 = nh * UNIT_W + h * 512
                        if USE_FP8:
                            nc.tensor.matmul(
                                pt[:, h * 512 : (h + 1) * 512],
                                lhsT=xT[:, :, m * 128 : (m + 1) * 128],
                                rhs=yT[:, :, c0 : c0 + 512],
                                start=True, stop=not is_act,
                                perf_mode=DR,
                            )
                        else:
                            nc.tensor.matmul(
                                pt[:, h * 512 : (h + 1) * 512],
                                lhsT=xT[:, m * 128 : (m + 1) * 128],
                                rhs=yT[:, c0 : c0 + 512],
                                start=True, stop=not is_act,
                            )
                    if is_act:
                        for h in range(HW):
                            c0 = nh * UNIT_W + h * 512
                            nc.tensor.matmul(
                                pt[:, h * 512 : (h + 1) * 512],
                                lhsT=negones[:],
                                rhs=brow[0:1, c0 : c0 + 512],
                                start=False, stop=True,
                            )
                        scr = epa.tile([128, UNIT_W], bf16, tag="ep_act")
                        nc.scalar.activation(
                            scr[:], pt[:], Act.Relu,
                            bias=acol[:, m : m + 1],
                            accum_out=res[:, col : col + 1],
                        )
                    else:
                        scr = epd.tile([128, UNIT_W], bf16, tag="ep_dve")
                        nc.vector.scalar_tensor_tensor(
                            out=scr[:], in0=pt[:], scalar=acol[:, m : m + 1],
                            in1=bb[:, nh * UNIT_W : (nh + 1) * UNIT_W],
                            op0=Alu.add, op1=Alu.max,
                            accum_out=res[:, col : col + 1],
                        )

            nc.scalar.dma_start(out_res[:], res[:])

    return nc


def kernel(x: np.ndarray, y: np.ndarray) -> np.ndarray:
    from concourse.bass_utils import run_bass_kernel_spmd
    import ml_dtypes

    x = np.ascontiguousarray(x, dtype=np.float32)
    y = np.ascontiguousarray(y, dtype=np.float32)

    if "nc" not in _cache:
        nc = _build()
        if not nc.is_finalized():
            nc.finalize()
        _cache["nc"] = nc
    nc = _cache["nc"]

    # host-side O(N*D) prologue
    yy = np.sum(y.astype(np.float64) * y.astype(np.float64), axis=1)
    b16 = yy.astype(np.float32).astype(ml_dtypes.bfloat16)
    z2 = 2.0 * np.sum(x.astype(np.float64) * y.astype(np.float64), axis=1)
    a = (yy - z2 + MARGIN).astype(np.float32)

    if USE_FP8:
        # [64, 2, cols] with contraction d = k2*64 + p, flattened to [64, 2*cols]
        xe = (2.0 * x).T.astype(ml_dtypes.float8_e4m3fn)  # [128, SH]
        ye = y.T.astype(ml_dtypes.float8_e4m3fn)          # [128, N]
        xtT = np.ascontiguousarray(
            xe.reshape(2, 64, N).transpose(1, 0, 2).reshape(64, 2 * N)
        )
        ytT = np.ascontiguousarray(
            ye.reshape(2, 64, N).transpose(1, 0, 2).reshape(64, 2 * N)
        )
    else:
        ytT = np.ascontiguousarray(y.T).astype(ml_dtypes.bfloat16)
        xtT = np.ascontiguousarray((2.0 * x).T).astype(ml_dtypes.bfloat16)

    bv = b16.reshape(1, N)
    in_maps = []
    for c in range(NCORES):
        sl = slice(c * SH, (c + 1) * SH)
        if USE_FP8:
            xc = xtT.reshape(64, 2, SH * NCORES)[:, :, sl].reshape(64, 2 * SH)
        else:
            xc = xtT[:, sl]
        in_maps.append({
            "xt": np.ascontiguousarray(xc),
            "yt": ytT,
            "bv": bv,
            "av": np.ascontiguousarray(a[sl].reshape(MT, 128).T),  # [128, MT]
        })

    _cache["in_maps"] = in_maps
    out = run_bass_kernel_spmd(nc, in_maps, list(range(NCORES)))
    results = out.results

    # host reduction (f64)
    total = 0.0
    for c in range(NCORES):
        total += np.asarray(results[c]["res"], dtype=np.float64).sum()
    bsum_blk = b16.astype(np.float64).reshape(NH, UNIT_W).sum(axis=1)
    # subtract Sum_b for every DVE (max-trick) unit
    for nh in range(NH):
        n_dve = sum(1 for m in range(MT) if not _is_act(m, nh))
        total -= NCORES * n_dve * 128.0 * bsum_blk[nh]
    total -= float(N) * float(np.float32(MARGIN))
    return np.float32(total / (float(N) * float(N)))


# revision 10
# speedup vs baseline: 1.3974x; 1.3091x over previous
"""HardTripletLoss (non-hardest branch) on 8 TRN2 NeuronCores — v2.3.

Math:  loss = mean_{i!=j} relu(d_pos[i] - pdist[i,j] + margin)
  pdist[i,j] = ||x_i||^2 + ||y_j||^2 - 2 x_i.y_j ,  d_pos = diag(pdist)
  =>  term(i,j) = relu(G[i,j] + a[i] - b[j])  with  G = 2 x y^T,
      a[i] = ||y_i||^2 - 2 x_i.y_i + margin,  b[j] = bf16(||y_j||^2).
Diagonal evaluates to ~relu(margin) = margin; host subtracts N*margin.

Host-side prep (O(N*D)): transpose + fp8/bf16-cast the matmul operands,
compute a and b.  Device does the O(N^2*D) matmul and the O(N^2)
relu/reduce epilogue.

G matmuls run in fp8 e4m3 DoubleRow mode (2 K-subtiles of 64 on 64
partitions -> 0.5 cycles/row).  Epilogue alternates DVE (max-trick) and
ACT (PE folds -b via K=1 bf16 matmuls, then Relu w/ bias=a) per m-tile.
Row-partial sums land in res [128, NU] per core; host reduces in f64.
"""

import sys

if "/opt/trn_rl_repo" not in sys.path:
    sys.path.insert(0, "/opt/trn_rl_repo")

import numpy as np

N, D = 8192, 128
NCORES = 8
SH = N // NCORES          # 1024 x-rows per core
MT = SH // 128            # 8 m-tiles
MARGIN = 0.2

UNIT_W = 1024             # epilogue tile width (psum: UNIT_W/512 banks)
NH = N // UNIT_W          # column blocks per row of m-tiles
NU = MT * NH              # total units per core
PS_BUFS = 8 * 512 // UNIT_W  # use all 8 psum banks
USE_FP8 = False


def _is_act(m, nh):
    # engine assignment: ACT (fold path) vs DVE (max-trick).
    # ACT units cost extra PE rows (the -b fold), so keep them to ~1/4.
    return (m + nh) % 4 == 3


_cache = {}


def _build():
    import concourse.mybir as mybir
    from concourse import bacc
    from concourse.tile import TileContext

    f32 = mybir.dt.float32
    bf16 = mybir.dt.bfloat16
    fp8 = mybir.dt.float8e4
    mmdt = fp8 if USE_FP8 else bf16
    Alu = mybir.AluOpType
    Act = mybir.ActivationFunctionType
    DR = mybir.MatmulPerfMode.DoubleRow

    nc = bacc.Bacc()
    # host-pre-transposed operands: xt = (2x)^T, yt = y^T
    # fp8 DoubleRow layout: [64, 2, cols], contraction d = k2*64 + p
    if USE_FP8:
        xt = nc.declare_dram_parameter("xt", [64, 2 * SH], mmdt, isOutput=False)
        yt = nc.declare_dram_parameter("yt", [64, 2 * N], mmdt, isOutput=False)
    else:
        xt = nc.declare_dram_parameter("xt", [128, SH], mmdt, isOutput=False)
        yt = nc.declare_dram_parameter("yt", [128, N], mmdt, isOutput=False)
    bv = nc.declare_dram_parameter("bv", [1, N], bf16, isOutput=False)    # b row
    av = nc.declare_dram_parameter("av", [128, MT], f32, isOutput=False)  # a cols
    out_res = nc.declare_dram_parameter("res", [128, NU], f32, isOutput=True)

    with TileContext(nc) as tc:
        with (
            tc.tile_pool(name="big", bufs=1) as big,
            tc.tile_pool(name="epd", bufs=3) as epd,
            tc.tile_pool(name="epa", bufs=3) as epa,
            tc.tile_pool(name="ps", bufs=PS_BUFS, space="PSUM") as ps,
        ):
            if USE_FP8:
                yT = big.tile([64, 2, N], mmdt)      # y^T  [p, k2, j]
                xT = big.tile([64, 2, SH], mmdt)     # (2x)^T [p, k2, i]
            else:
                yT = big.tile([128, N], mmdt)
                xT = big.tile([128, SH], mmdt)
            bb = big.tile([128, N], bf16)            # b broadcast to all partitions
            brow = big.tile([1, N], bf16)            # b as a single row
            acol = big.tile([128, MT], f32)          # a per m-tile column
            negones = big.tile([1, 128], bf16)
            res = big.tile([128, NU], f32)

            nc.vector.memset(negones[:], -1.0)

            # ---- loads: yt chunks on sync ring; the rest on scalar ring ----
            for q in range(4):
                if USE_FP8:
                    nc.sync.dma_start(
                        yT[:, :, q * 2048 : (q + 1) * 2048],
                        yt[:, :].rearrange("p (k c) -> p k c", k=2)[
                            :, :, q * 2048 : (q + 1) * 2048
                        ],
                    )
                else:
                    nc.sync.dma_start(
                        yT[:, q * 2048 : (q + 1) * 2048],
                        yt[:, q * 2048 : (q + 1) * 2048],
                    )
            if USE_FP8:
                nc.scalar.dma_start(
                    xT[:], xt[:, :].rearrange("p (k c) -> p k c", k=2)
                )
            else:
                nc.scalar.dma_start(xT[:], xt[:, :])
            nc.scalar.dma_start(bb[:], bv[0:1, :].broadcast_to([128, N]))
            nc.scalar.dma_start(acol[:], av[:, :])
            nc.scalar.dma_start(brow[:], bv[:, :])

            # ---- main: G tiles + fused epilogue ----
            HW = UNIT_W // 512  # matmuls per unit
            for nh in range(NH):
                for m in range(MT):
                    col = m * NH + nh  # res column (m-major for host)
                    is_act = _is_act(m, nh)
                    pt = ps.tile([128, UNIT_W], f32, tag="g")
                    for h in range(HW):
                        c0 = nh * UNIT_W + h * 512
                        if USE_FP8:
                            nc.tensor.matmul(
                                pt[:, h * 512 : (h + 1) * 512],
                                lhsT=xT[:, :, m * 128 : (m + 1) * 128],
                                rhs=yT[:, :, c0 : c0 + 512],
                                start=True, stop=not is_act,
                                perf_mode=DR,
                            )
                        else:
                            nc.tensor.matmul(
                                pt[:, h * 512 : (h + 1) * 512],
                                lhsT=xT[:, m * 128 : (m + 1) * 128],
                                rhs=yT[:, c0 : c0 + 512],
                                start=True, stop=not is_act,
                            )
                    if is_act:
                        for h in range(HW):
                            c0 = nh * UNIT_W + h * 512
                            nc.tensor.matmul(
                                pt[:, h * 512 : (h + 1) * 512],
                                lhsT=negones[:],
                                rhs=brow[0:1, c0 : c0 + 512],
                                start=False, stop=True,
                            )
                        scr = epa.tile([128, UNIT_W], bf16, tag="ep_act")
                        nc.scalar.activation(
                            scr[:], pt[:], Act.Relu,
                            bias=acol[:, m : m + 1],
                            accum_out=res[:, col : col + 1],
                        )
                    else:
                        scr = epd.tile([128, UNIT_W], bf16, tag="ep_dve")
                        nc.vector.scalar_tensor_tensor(
                            out=scr[:], in0=pt[:], scalar=acol[:, m : m + 1],
                            in1=bb[:, nh * UNIT_W : (nh + 1) * UNIT_W],
                            op0=Alu.add, op1=Alu.max,
                            accum_out=res[:, col : col + 1],
                        )

            nc.scalar.dma_start(out_res[:], res[:])

    return nc


def kernel(x: np.ndarray, y: np.ndarray) -> np.ndarray:
    from concourse.bass_utils import run_bass_kernel_spmd
    import ml_dtypes

    x = np.ascontiguousarray(x, dtype=np.float32)
    y = np.ascontiguousarray(y, dtype=np.float32)

    if "nc" not in _cache:
        nc = _build()
        if not nc.is_finalized():
            nc.finalize()
        _cache["nc"] = nc
    nc = _cache["nc"]

    # host-side O(N*D) prologue
    yy = np.sum(y.astype(np.float64) * y.astype(np.float64), axis=1)
    b16 = yy.astype(np.float32).astype(ml_dtypes.bfloat16)
    z2 = 2.0 * np.sum(x.astype(np.float64) * y.astype(np.float64), axis=1)
    a = (yy - z2 + MARGIN).astype(np.float32)

    if USE_FP8:
        # [64, 2, cols] with contraction d = k2*64 + p, flattened to [64, 2*cols]
        xe = (2.0 * x).T.astype(ml_dtypes.float8_e4m3fn)  # [128, SH]
        ye = y.T.astype(ml_dtypes.float8_e4m3fn)          # [128, N]
        xtT = np.ascontiguousarray(
            xe.reshape(2, 64, N).transpose(1, 0, 2).reshape(64, 2 * N)
        )
        ytT = np.ascontiguousarray(
            ye.reshape(2, 64, N).transpose(1, 0, 2).reshape(64, 2 * N)
        )
    else:
        ytT = np.ascontiguousarray(y.T).astype(ml_dtypes.bfloat16)
        xtT = np.ascontiguousarray((2.0 * x).T).astype(ml_dtypes.bfloat16)

    bv = b16.reshape(1, N)
    in_maps = []
    for c in range(NCORES):
        sl = slice(c * SH, (c + 1) * SH)
        if USE_FP8:
            xc = xtT.reshape(64, 2, SH * NCORES)[:, :, sl].reshape(64, 2 * SH)
        else:
            xc = xtT[:, sl]
        in_maps.append({
            "xt": np.ascontiguousarray(xc),
            "yt": ytT,
            "bv": bv,
            "av": np.ascontiguousarray(a[sl].reshape(MT, 128).T),  # [128, MT]
        })

    _cache["in_maps"] = in_maps
    out = run_bass_kernel_spmd(nc, in_maps, list(range(NCORES)))
    results = out.results

    # host reduction (f64)
    total = 0.0
    for c in range(NCORES):
        total += np.asarray(results[c]["res"], dtype=np.float64).sum()
    bsum_blk = b16.astype(np.float64).reshape(NH, UNIT_W).sum(axis=1)
    # subtract Sum_b for every DVE (max-trick) unit
    for nh in range(NH):
        n_dve = sum(1 for m in range(MT) if not _is_act(m, nh))
        total -= NCORES * n_dve * 128.0 * bsum_blk[nh]
    total -= float(N) * float(np.float32(MARGIN))
    return np.float32(total / (float(N) * float(N)))


# revision 13
# speedup vs baseline: 1.4618x; 1.0461x over previous
"""HardTripletLoss (non-hardest branch) on 8 TRN2 NeuronCores — v2.3.

Math:  loss = mean_{i!=j} relu(d_pos[i] - pdist[i,j] + margin)
  pdist[i,j] = ||x_i||^2 + ||y_j||^2 - 2 x_i.y_j ,  d_pos = diag(pdist)
  =>  term(i,j) = relu(G[i,j] + a[i] - b[j])  with  G = 2 x y^T,
      a[i] = ||y_i||^2 - 2 x_i.y_i + margin,  b[j] = bf16(||y_j||^2).
Diagonal evaluates to ~relu(margin) = margin; host subtracts N*margin.

Host-side prep (O(N*D)): transpose + fp8/bf16-cast the matmul operands,
compute a and b.  Device does the O(N^2*D) matmul and the O(N^2)
relu/reduce epilogue.

G matmuls run in fp8 e4m3 DoubleRow mode (2 K-subtiles of 64 on 64
partitions -> 0.5 cycles/row).  Epilogue alternates DVE (max-trick) and
ACT (PE folds -b via K=1 bf16 matmuls, then Relu w/ bias=a) per m-tile.
Row-partial sums land in res [128, NU] per core; host reduces in f64.
"""

import sys

if "/opt/trn_rl_repo" not in sys.path:
    sys.path.insert(0, "/opt/trn_rl_repo")

import numpy as np

N, D = 8192, 128
NCORES = 8
SH = N // NCORES          # 1024 x-rows per core
MT = SH // 128            # 8 m-tiles
MARGIN = 0.2

UNIT_W = 1024             # epilogue tile width (psum: UNIT_W/512 banks)
NH = N // UNIT_W          # column blocks per row of m-tiles
NU = MT * NH              # total units per core
PS_BUFS = 8 * 512 // UNIT_W  # use all 8 psum banks
USE_FP8 = False


def _is_act(m, nh):
    # engine assignment: ACT (fold path) vs DVE (max-trick).
    # ACT units cost extra PE rows (the -b fold); 12/64 balances
    # PE ~= (128 + 2a)*0.43us against DVE ~= (64-a)*1.28us.
    if nh % 2 == 0:
        return m in (2, 5)
    return m == 4


_cache = {}


def _build():
    import concourse.mybir as mybir
    from concourse import bacc
    from concourse.tile import TileContext

    f32 = mybir.dt.float32
    bf16 = mybir.dt.bfloat16
    fp8 = mybir.dt.float8e4
    mmdt = fp8 if USE_FP8 else bf16
    Alu = mybir.AluOpType
    Act = mybir.ActivationFunctionType
    DR = mybir.MatmulPerfMode.DoubleRow

    nc = bacc.Bacc()
    # host-pre-transposed operands: xt = (2x)^T, yt = y^T
    # fp8 DoubleRow layout: [64, 2, cols], contraction d = k2*64 + p
    if USE_FP8:
        xt = nc.declare_dram_parameter("xt", [64, 2 * SH], mmdt, isOutput=False)
        yt = nc.declare_dram_parameter("yt", [64, 2 * N], mmdt, isOutput=False)
    else:
        xt = nc.declare_dram_parameter("xt", [128, SH], mmdt, isOutput=False)
        yt = nc.declare_dram_parameter("yt", [128, N], mmdt, isOutput=False)
    bv = nc.declare_dram_parameter("bv", [1, N], bf16, isOutput=False)    # b row
    av = nc.declare_dram_parameter("av", [128, MT], f32, isOutput=False)  # a cols
    out_res = nc.declare_dram_parameter("res", [128, NU], f32, isOutput=True)

    with TileContext(nc) as tc:
        with (
            tc.tile_pool(name="big", bufs=1) as big,
            tc.tile_pool(name="epd", bufs=3) as epd,
            tc.tile_pool(name="epa", bufs=3) as epa,
            tc.tile_pool(name="ps", bufs=PS_BUFS, space="PSUM") as ps,
        ):
            if USE_FP8:
                yT = big.tile([64, 2, N], mmdt)      # y^T  [p, k2, j]
                xT = big.tile([64, 2, SH], mmdt)     # (2x)^T [p, k2, i]
            else:
                yT = big.tile([128, N], mmdt)
                xT = big.tile([128, SH], mmdt)
            bb = big.tile([128, N], bf16)            # b broadcast to all partitions
            brow = big.tile([1, N], bf16)            # b as a single row
            acol = big.tile([128, MT], f32)          # a per m-tile column
            negones = big.tile([1, 128], bf16)
            res = big.tile([128, NU], f32)

            nc.vector.memset(negones[:], -1.0)

            # ---- loads: yt chunks on sync ring; the rest on scalar ring ----
            # small leading chunks so the first matmuls / first DVE unit
            # start as early as possible.
            for c0, c1 in ((0, 1024), (1024, 2048), (2048, 4096), (4096, 8192)):
                if USE_FP8:
                    nc.sync.dma_start(
                        yT[:, :, c0:c1],
                        yt[:, :].rearrange("p (k c) -> p k c", k=2)[:, :, c0:c1],
                    )
                else:
                    nc.sync.dma_start(yT[:, c0:c1], yt[:, c0:c1])
            if USE_FP8:
                nc.scalar.dma_start(
                    xT[:], xt[:, :].rearrange("p (k c) -> p k c", k=2)
                )
            else:
                nc.scalar.dma_start(xT[:], xt[:, :])
            nc.scalar.dma_start(acol[:], av[:, :])
            nc.scalar.dma_start(brow[:], bv[:, :])
            nc.scalar.dma_start(
                bb[:, 0:1024], bv[0:1, 0:1024].broadcast_to([128, 1024])
            )
            nc.scalar.dma_start(
                bb[:, 1024:N], bv[0:1, 1024:N].broadcast_to([128, N - 1024])
            )

            # ---- main: G tiles + fused epilogue ----
            HW = UNIT_W // 512  # matmuls per unit
            for nh in range(NH):
                # ACT units first within each block: they don't need the
                # (large, late) bb broadcast
                ms = [m for m in range(MT) if _is_act(m, nh)] + [
                    m for m in range(MT) if not _is_act(m, nh)
                ]
                for m in ms:
                    col = m * NH + nh  # res column (m-major for host)
                    is_act = _is_act(m, nh)
                    pt = ps.tile([128, UNIT_W], f32, tag="g")
                    for h in range(HW):
                        c0 = nh * UNIT_W + h * 512
                        if USE_FP8:
                            nc.tensor.matmul(
                                pt[:, h * 512 : (h + 1) * 512],
                                lhsT=xT[:, :, m * 128 : (m + 1) * 128],
                                rhs=yT[:, :, c0 : c0 + 512],
                                start=True, stop=not is_act,
                                perf_mode=DR,
                            )
                        else:
                            nc.tensor.matmul(
                                pt[:, h * 512 : (h + 1) * 512],
                                lhsT=xT[:, m * 128 : (m + 1) * 128],
                                rhs=yT[:, c0 : c0 + 512],
                                start=True, stop=not is_act,
                            )
                    if is_act:
                        for h in range(HW):
                            c0 = nh * UNIT_W + h * 512
                            nc.tensor.matmul(
                                pt[:, h * 512 : (h + 1) * 512],
                                lhsT=negones[:],
                                rhs=brow[0:1, c0 : c0 + 512],
                                start=False, stop=True,
                            )
                        scr = epa.tile([128, UNIT_W], bf16, tag="ep_act")
                        nc.scalar.activation(
                            scr[:], pt[:], Act.Relu,
                            bias=acol[:, m : m + 1],
                            accum_out=res[:, col : col + 1],
                        )
                    else:
                        scr = epd.tile([128, UNIT_W], bf16, tag="ep_dve")
                        nc.vector.scalar_tensor_tensor(
                            out=scr[:], in0=pt[:], scalar=acol[:, m : m + 1],
                            in1=bb[:, nh * UNIT_W : (nh + 1) * UNIT_W],
                            op0=Alu.add, op1=Alu.max,
                            accum_out=res[:, col : col + 1],
                        )

            nc.scalar.dma_start(out_res[:], res[:])

    return nc


def kernel(x: np.ndarray, y: np.ndarray) -> np.ndarray:
    from concourse.bass_utils import run_bass_kernel_spmd
    import ml_dtypes

    x = np.ascontiguousarray(x, dtype=np.float32)
    y = np.ascontiguousarray(y, dtype=np.float32)

    if "nc" not in _cache:
        nc = _build()
        if not nc.is_finalized():
            nc.finalize()
        _cache["nc"] = nc
    nc = _cache["nc"]

    # host-side O(N*D) prologue
    yy = np.sum(y.astype(np.float64) * y.astype(np.float64), axis=1)
    b16 = yy.astype(np.float32).astype(ml_dtypes.bfloat16)
    z2 = 2.0 * np.sum(x.astype(np.float64) * y.astype(np.float64), axis=1)
    a = (yy - z2 + MARGIN).astype(np.float32)

    if USE_FP8:
        # [64, 2, cols] with contraction d = k2*64 + p, flattened to [64, 2*cols]
        xe = (2.0 * x).T.astype(ml_dtypes.float8_e4m3fn)  # [128, SH]
        ye = y.T.astype(ml_dtypes.float8_e4m3fn)          # [128, N]
        xtT = np.ascontiguousarray(
            xe.reshape(2, 64, N).transpose(1, 0, 2).reshape(64, 2 * N)
        )
        ytT = np.ascontiguousarray(
            ye.reshape(2, 64, N).transpose(1, 0, 2).reshape(64, 2 * N)
        )
    else:
        ytT = np.ascontiguousarray(y.T).astype(ml_dtypes.bfloat16)
        xtT = np.ascontiguousarray((2.0 * x).T).astype(ml_dtypes.bfloat16)

    bv = b16.reshape(1, N)
    in_maps = []
    for c in range(NCORES):
        sl = slice(c * SH, (c + 1) * SH)
        if USE_FP8:
            xc = xtT.reshape(64, 2, SH * NCORES)[:, :, sl].reshape(64, 2 * SH)
        else:
            xc = xtT[:, sl]
        in_maps.append({
            "xt": np.ascontiguousarray(xc),
            "yt": ytT,
            "bv": bv,
            "av": np.ascontiguousarray(a[sl].reshape(MT, 128).T),  # [128, MT]
        })

    _cache["in_maps"] = in_maps
    out = run_bass_kernel_spmd(nc, in_maps, list(range(NCORES)))
    results = out.results

    # host reduction (f64)
    total = 0.0
    for c in range(NCORES):
        total += np.asarray(results[c]["res"], dtype=np.float64).sum()
    bsum_blk = b16.astype(np.float64).reshape(NH, UNIT_W).sum(axis=1)
    # subtract Sum_b for every DVE (max-trick) unit
    for nh in range(NH):
        n_dve = sum(1 for m in range(MT) if not _is_act(m, nh))
        total -= NCORES * n_dve * 128.0 * bsum_blk[nh]
    total -= float(N) * float(np.float32(MARGIN))
    return np.float32(total / (float(N) * float(N)))


# revision 15
# speedup vs baseline: 1.4934x; 1.0216x over previous
"""HardTripletLoss (non-hardest branch) on 8 TRN2 NeuronCores — v2.3.

Math:  loss = mean_{i!=j} relu(d_pos[i] - pdist[i,j] + margin)
  pdist[i,j] = ||x_i||^2 + ||y_j||^2 - 2 x_i.y_j ,  d_pos = diag(pdist)
  =>  term(i,j) = relu(G[i,j] + a[i] - b[j])  with  G = 2 x y^T,
      a[i] = ||y_i||^2 - 2 x_i.y_i + margin,  b[j] = bf16(||y_j||^2).
Diagonal evaluates to ~relu(margin) = margin; host subtracts N*margin.

Host-side prep (O(N*D)): transpose + fp8/bf16-cast the matmul operands,
compute a and b.  Device does the O(N^2*D) matmul and the O(N^2)
relu/reduce epilogue.

G matmuls run in fp8 e4m3 DoubleRow mode (2 K-subtiles of 64 on 64
partitions -> 0.5 cycles/row).  Epilogue alternates DVE (max-trick) and
ACT (PE folds -b via K=1 bf16 matmuls, then Relu w/ bias=a) per m-tile.
Row-partial sums land in res [128, NU] per core; host reduces in f64.
"""

import sys

if "/opt/trn_rl_repo" not in sys.path:
    sys.path.insert(0, "/opt/trn_rl_repo")

import numpy as np

N, D = 8192, 128
NCORES = 8
SH = N // NCORES          # 1024 x-rows per core
MT = SH // 128            # 8 m-tiles
MARGIN = 0.2

UNIT_W = 1024             # epilogue tile width (psum: UNIT_W/512 banks)
NH = N // UNIT_W          # column blocks per row of m-tiles
NU = MT * NH              # total units per core
PS_BUFS = 8 * 512 // UNIT_W  # use all 8 psum banks
USE_FP8 = False


def _is_act(m, nh):
    # engine assignment: ACT (fold path) vs DVE (max-trick).
    # ACT units cost extra PE rows (the -b fold); ~12/64 balances
    # PE ~= (128 + 2a)*0.43us against DVE ~= (64-a)*1.28us.  The last
    # block leans ACT so the engines finish together (ACT/PE idle at
    # the end while DVE drains).
    if nh == 7:
        return m in (1, 4, 6)
    if nh % 2 == 0:
        return m in (2, 5)
    return m == 4


_cache = {}


def _build():
    import concourse.mybir as mybir
    from concourse import bacc
    from concourse.tile import TileContext

    f32 = mybir.dt.float32
    bf16 = mybir.dt.bfloat16
    fp8 = mybir.dt.float8e4
    mmdt = fp8 if USE_FP8 else bf16
    Alu = mybir.AluOpType
    Act = mybir.ActivationFunctionType
    DR = mybir.MatmulPerfMode.DoubleRow

    nc = bacc.Bacc()
    # host-pre-transposed operands: xt = (2x)^T, yt = y^T
    # fp8 DoubleRow layout: [64, 2, cols], contraction d = k2*64 + p
    if USE_FP8:
        xt = nc.declare_dram_parameter("xt", [64, 2 * SH], mmdt, isOutput=False)
        yt = nc.declare_dram_parameter("yt", [64, 2 * N], mmdt, isOutput=False)
    else:
        xt = nc.declare_dram_parameter("xt", [128, SH], mmdt, isOutput=False)
        yt = nc.declare_dram_parameter("yt", [128, N], mmdt, isOutput=False)
    bv = nc.declare_dram_parameter("bv", [1, N], bf16, isOutput=False)    # b row
    av = nc.declare_dram_parameter("av", [128, MT], f32, isOutput=False)  # a cols
    out_res = nc.declare_dram_parameter("res", [128, NU], f32, isOutput=True)

    with TileContext(nc) as tc:
        with (
            tc.tile_pool(name="big", bufs=1) as big,
            tc.tile_pool(name="epd", bufs=3) as epd,
            tc.tile_pool(name="epa", bufs=3) as epa,
            tc.tile_pool(name="ps", bufs=PS_BUFS, space="PSUM") as ps,
        ):
            if USE_FP8:
                yT = big.tile([64, 2, N], mmdt)      # y^T  [p, k2, j]
                xT = big.tile([64, 2, SH], mmdt)     # (2x)^T [p, k2, i]
            else:
                yT = big.tile([128, N], mmdt)
                xT = big.tile([128, SH], mmdt)
            bb = big.tile([128, N], bf16)            # b broadcast to all partitions
            brow = big.tile([1, N], bf16)            # b as a single row
            acol = big.tile([128, MT], f32)          # a per m-tile column
            negones = big.tile([1, 128], bf16)
            res = big.tile([128, NU], f32)

            nc.vector.memset(negones[:], -1.0)

            # ---- loads: yt chunks on sync ring; the rest on scalar ring ----
            # small leading chunks so the first matmuls / first DVE unit
            # start as early as possible.
            for c0, c1 in ((0, 1024), (1024, 2048), (2048, 4096), (4096, 8192)):
                if USE_FP8:
                    nc.sync.dma_start(
                        yT[:, :, c0:c1],
                        yt[:, :].rearrange("p (k c) -> p k c", k=2)[:, :, c0:c1],
                    )
                else:
                    nc.sync.dma_start(yT[:, c0:c1], yt[:, c0:c1])
            if USE_FP8:
                nc.scalar.dma_start(
                    xT[:], xt[:, :].rearrange("p (k c) -> p k c", k=2)
                )
            else:
                nc.scalar.dma_start(xT[:], xt[:, :])
            nc.scalar.dma_start(acol[:], av[:, :])
            nc.scalar.dma_start(brow[:], bv[:, :])
            nc.scalar.dma_start(
                bb[:, 0:1024], bv[0:1, 0:1024].broadcast_to([128, 1024])
            )
            nc.scalar.dma_start(
                bb[:, 1024:N], bv[0:1, 1024:N].broadcast_to([128, N - 1024])
            )

            # ---- main: G tiles + fused epilogue ----
            HW = UNIT_W // 512  # matmuls per unit
            for nh in range(NH):
                # interleave ACT units among DVE units, ACT leading (they
                # don't need the large, late bb broadcast)
                act_ms = [m for m in range(MT) if _is_act(m, nh)]
                dve_ms = [m for m in range(MT) if not _is_act(m, nh)]
                ms = []
                stride = max(1, len(dve_ms) // max(1, len(act_ms)))
                di = 0
                for am in act_ms:
                    ms.append(am)
                    ms.extend(dve_ms[di : di + stride])
                    di += stride
                ms.extend(dve_ms[di:])
                for m in ms:
                    col = m * NH + nh  # res column (m-major for host)
                    is_act = _is_act(m, nh)
                    pt = ps.tile([128, UNIT_W], f32, tag="g")
                    for h in range(HW):
                        c0 = nh * UNIT_W + h * 512
                        if USE_FP8:
                            nc.tensor.matmul(
                                pt[:, h * 512 : (h + 1) * 512],
                                lhsT=xT[:, :, m * 128 : (m + 1) * 128],
                                rhs=yT[:, :, c0 : c0 + 512],
                                start=True, stop=not is_act,
                                perf_mode=DR,
                            )
                        else:
                            nc.tensor.matmul(
                                pt[:, h * 512 : (h + 1) * 512],
                                lhsT=xT[:, m * 128 : (m + 1) * 128],
                                rhs=yT[:, c0 : c0 + 512],
                                start=True, stop=not is_act,
                            )
                    if is_act:
                        for h in range(HW):
                            c0 = nh * UNIT_W + h * 512
                            nc.tensor.matmul(
                                pt[:, h * 512 : (h + 1) * 512],
                                lhsT=negones[:],
                                rhs=brow[0:1, c0 : c0 + 512],
                                start=False, stop=True,
                            )
                        scr = epa.tile([128, UNIT_W], bf16, tag="ep_act")
                        nc.scalar.activation(
                            scr[:], pt[:], Act.Relu,
                            bias=acol[:, m : m + 1],
                            accum_out=res[:, col : col + 1],
                        )
                    else:
                        scr = epd.tile([128, UNIT_W], bf16, tag="ep_dve")
                        nc.vector.scalar_tensor_tensor(
                            out=scr[:], in0=pt[:], scalar=acol[:, m : m + 1],
                            in1=bb[:, nh * UNIT_W : (nh + 1) * UNIT_W],
                            op0=Alu.add, op1=Alu.max,
                            accum_out=res[:, col : col + 1],
                        )

            nc.scalar.dma_start(out_res[:], res[:])

    return nc


def kernel(x: np.ndarray, y: np.ndarray) -> np.ndarray:
    from concourse.bass_utils import run_bass_kernel_spmd
    import ml_dtypes

    x = np.ascontiguousarray(x, dtype=np.float32)
    y = np.ascontiguousarray(y, dtype=np.float32)

    if "nc" not in _cache:
        nc = _build()
        if not nc.is_finalized():
            nc.finalize()
        _cache["nc"] = nc
    nc = _cache["nc"]

    # host-side O(N*D) prologue
    yy = np.sum(y.astype(np.float64) * y.astype(np.float64), axis=1)
    b16 = yy.astype(np.float32).astype(ml_dtypes.bfloat16)
    z2 = 2.0 * np.sum(x.astype(np.float64) * y.astype(np.float64), axis=1)
    a = (yy - z2 + MARGIN).astype(np.float32)

    if USE_FP8:
        # [64, 2, cols] with contraction d = k2*64 + p, flattened to [64, 2*cols]
        xe = (2.0 * x).T.astype(ml_dtypes.float8_e4m3fn)  # [128, SH]
        ye = y.T.astype(ml_dtypes.float8_e4m3fn)          # [128, N]
        xtT = np.ascontiguousarray(
            xe.reshape(2, 64, N).transpose(1, 0, 2).reshape(64, 2 * N)
        )
        ytT = np.ascontiguousarray(
            ye.reshape(2, 64, N).transpose(1, 0, 2).reshape(64, 2 * N)
        )
    else:
        ytT = np.ascontiguousarray(y.T).astype(ml_dtypes.bfloat16)
        xtT = np.ascontiguousarray((2.0 * x).T).astype(ml_dtypes.bfloat16)

    bv = b16.reshape(1, N)
    in_maps = []
    for c in range(NCORES):
        sl = slice(c * SH, (c + 1) * SH)
        if USE_FP8:
            xc = xtT.reshape(64, 2, SH * NCORES)[:, :, sl].reshape(64, 2 * SH)
        else:
            xc = xtT[:, sl]
        in_maps.append({
            "xt": np.ascontiguousarray(xc),
            "yt": ytT,
            "bv": bv,
            "av": np.ascontiguousarray(a[sl].reshape(MT, 128).T),  # [128, MT]
        })

    _cache["in_maps"] = in_maps
    out = run_bass_kernel_spmd(nc, in_maps, list(range(NCORES)))
    results = out.results

    # host reduction (f64)
    total = 0.0
    for c in range(NCORES):
        total += np.asarray(results[c]["res"], dtype=np.float64).sum()
    bsum_blk = b16.astype(np.float64).reshape(NH, UNIT_W).sum(axis=1)
    # subtract Sum_b for every DVE (max-trick) unit
    for nh in range(NH):
        n_dve = sum(1 for m in range(MT) if not _is_act(m, nh))
        total -= NCORES * n_dve * 128.0 * bsum_blk[nh]
    total -= float(N) * float(np.float32(MARGIN))
    return np.float32(total / (float(N) * float(N)))


# revision 16
# speedup vs baseline: 1.5729x; 1.0533x over previous
"""HardTripletLoss (non-hardest branch) on 8 TRN2 NeuronCores — v2.3.

Math:  loss = mean_{i!=j} relu(d_pos[i] - pdist[i,j] + margin)
  pdist[i,j] = ||x_i||^2 + ||y_j||^2 - 2 x_i.y_j ,  d_pos = diag(pdist)
  =>  term(i,j) = relu(G[i,j] + a[i] - b[j])  with  G = 2 x y^T,
      a[i] = ||y_i||^2 - 2 x_i.y_i + margin,  b[j] = bf16(||y_j||^2).
Diagonal evaluates to ~relu(margin) = margin; host subtracts N*margin.

Host-side prep (O(N*D)): transpose + fp8/bf16-cast the matmul operands,
compute a and b.  Device does the O(N^2*D) matmul and the O(N^2)
relu/reduce epilogue.

G matmuls run in fp8 e4m3 DoubleRow mode (2 K-subtiles of 64 on 64
partitions -> 0.5 cycles/row).  Epilogue alternates DVE (max-trick) and
ACT (PE folds -b via K=1 bf16 matmuls, then Relu w/ bias=a) per m-tile.
Row-partial sums land in res [128, NU] per core; host reduces in f64.
"""

import sys

if "/opt/trn_rl_repo" not in sys.path:
    sys.path.insert(0, "/opt/trn_rl_repo")

import numpy as np

N, D = 8192, 128
NCORES = 8
SH = N // NCORES          # 1024 x-rows per core
MT = SH // 128            # 8 m-tiles
MARGIN = 0.2

UNIT_W = 1024             # epilogue tile width (psum: UNIT_W/512 banks)
NH = N // UNIT_W          # column blocks per row of m-tiles
NU = MT * NH              # total units per core
PS_BUFS = 8 * 512 // UNIT_W  # use all 8 psum banks
USE_FP8 = False


def _is_act(m, nh):
    # engine assignment: ACT (fold path) vs DVE (max-trick).
    # ACT units cost extra PE rows (the -b fold); ~12/64 balances
    # PE ~= (128 + 2a)*0.43us against DVE ~= (64-a)*1.28us.  The last
    # block leans ACT so the engines finish together (ACT/PE idle at
    # the end while DVE drains).
    if nh == 7:
        return m in (1, 4, 6)
    if nh % 2 == 0:
        return m in (2, 5)
    return m == 4


_cache = {}


def _build():
    import concourse.mybir as mybir
    from concourse import bacc
    from concourse.tile import TileContext

    f32 = mybir.dt.float32
    bf16 = mybir.dt.bfloat16
    fp8 = mybir.dt.float8e4
    mmdt = fp8 if USE_FP8 else bf16
    Alu = mybir.AluOpType
    Act = mybir.ActivationFunctionType
    DR = mybir.MatmulPerfMode.DoubleRow

    nc = bacc.Bacc()
    # host-pre-transposed operands: xt = (2x)^T, yt = y^T
    # fp8 DoubleRow layout: [64, 2, cols], contraction d = k2*64 + p
    if USE_FP8:
        xt = nc.declare_dram_parameter("xt", [64, 2 * SH], mmdt, isOutput=False)
        yt = nc.declare_dram_parameter("yt", [64, 2 * N], mmdt, isOutput=False)
    else:
        xt = nc.declare_dram_parameter("xt", [128, SH], mmdt, isOutput=False)
        yt = nc.declare_dram_parameter("yt", [128, N], mmdt, isOutput=False)
    bv = nc.declare_dram_parameter("bv", [1, N], bf16, isOutput=False)    # b row
    av = nc.declare_dram_parameter("av", [128, MT], f32, isOutput=False)  # a cols
    out_res = nc.declare_dram_parameter("res", [128, NU], f32, isOutput=True)

    with TileContext(nc) as tc:
        with (
            tc.tile_pool(name="big", bufs=1) as big,
            tc.tile_pool(name="epd", bufs=3) as epd,
            tc.tile_pool(name="epa", bufs=3) as epa,
            tc.tile_pool(name="ps", bufs=PS_BUFS, space="PSUM") as ps,
        ):
            if USE_FP8:
                yT = big.tile([64, 2, N], mmdt)      # y^T  [p, k2, j]
                xT = big.tile([64, 2, SH], mmdt)     # (2x)^T [p, k2, i]
            else:
                yT = big.tile([128, N], mmdt)
                xT = big.tile([128, SH], mmdt)
            bb = big.tile([128, N], bf16)            # b broadcast to all partitions
            brow = big.tile([1, N], bf16)            # b as a single row
            acol = big.tile([128, MT], f32)          # a per m-tile column
            negones = big.tile([1, 128], bf16)
            res = big.tile([128, NU], f32)

            nc.vector.memset(negones[:], -1.0)

            # ---- loads: yt chunks on sync ring; the rest on scalar ring ----
            # small leading chunks so the first matmuls / first DVE unit
            # start as early as possible.
            def _bcast(c0, c1):
                return bv[0:1, c0:c1].broadcast_to([128, c1 - c0])

            # sync ring: yt chunks interleaved with early bb chunks
            nc.sync.dma_start(yT[:, 0:1024], yt[:, 0:1024])
            nc.sync.dma_start(bb[:, 0:2048], _bcast(0, 2048))
            nc.sync.dma_start(yT[:, 1024:4096], yt[:, 1024:4096])
            nc.sync.dma_start(bb[:, 2048:4096], _bcast(2048, 4096))
            nc.sync.dma_start(yT[:, 4096:N], yt[:, 4096:N])
            # scalar ring: x operand + small vectors + late bb half
            nc.scalar.dma_start(xT[:], xt[:, :])
            nc.scalar.dma_start(acol[:], av[:, :])
            nc.scalar.dma_start(brow[:], bv[:, :])
            nc.scalar.dma_start(bb[:, 4096:N], _bcast(4096, N))

            # ---- main: G tiles + fused epilogue ----
            HW = UNIT_W // 512  # matmuls per unit
            for nh in range(NH):
                # interleave ACT units among DVE units, ACT leading (they
                # don't need the large, late bb broadcast)
                act_ms = [m for m in range(MT) if _is_act(m, nh)]
                dve_ms = [m for m in range(MT) if not _is_act(m, nh)]
                ms = []
                stride = max(1, len(dve_ms) // max(1, len(act_ms)))
                di = 0
                for am in act_ms:
                    ms.append(am)
                    ms.extend(dve_ms[di : di + stride])
                    di += stride
                ms.extend(dve_ms[di:])
                for m in ms:
                    col = m * NH + nh  # res column (m-major for host)
                    is_act = _is_act(m, nh)
                    pt = ps.tile([128, UNIT_W], f32, tag="g")
                    for h in range(HW):
                        c0 = nh * UNIT_W + h * 512
                        if USE_FP8:
                            nc.tensor.matmul(
                                pt[:, h * 512 : (h + 1) * 512],
                                lhsT=xT[:, :, m * 128 : (m + 1) * 128],
                                rhs=yT[:, :, c0 : c0 + 512],
                                start=True, stop=not is_act,
                                perf_mode=DR,
                            )
                        else:
                            nc.tensor.matmul(
                                pt[:, h * 512 : (h + 1) * 512],
                                lhsT=xT[:, m * 128 : (m + 1) * 128],
                                rhs=yT[:, c0 : c0 + 512],
                                start=True, stop=not is_act,
                            )
                    if is_act:
                        for h in range(HW):
                            c0 = nh * UNIT_W + h * 512
                            nc.tensor.matmul(
                                pt[:, h * 512 : (h + 1) * 512],
                                lhsT=negones[:],
                                rhs=brow[0:1, c0 : c0 + 512],
                                start=False, stop=True,
                            )
                        scr = epa.tile([128, UNIT_W], bf16, tag="ep_act")
                        nc.scalar.activation(
                            scr[:], pt[:], Act.Relu,
                            bias=acol[:, m : m + 1],
                            accum_out=res[:, col : col + 1],
                        )
                    else:
                        scr = epd.tile([128, UNIT_W], bf16, tag="ep_dve")
                        nc.vector.scalar_tensor_tensor(
                            out=scr[:], in0=pt[:], scalar=acol[:, m : m + 1],
                            in1=bb[:, nh * UNIT_W : (nh + 1) * UNIT_W],
                            op0=Alu.add, op1=Alu.max,
                            accum_out=res[:, col : col + 1],
                        )

            nc.scalar.dma_start(out_res[:], res[:])

    return nc


def kernel(x: np.ndarray, y: np.ndarray) -> np.ndarray:
    from concourse.bass_utils import run_bass_kernel_spmd
    import ml_dtypes

    x = np.ascontiguousarray(x, dtype=np.float32)
    y = np.ascontiguousarray(y, dtype=np.float32)

    if "nc" not in _cache:
        nc = _build()
        if not nc.is_finalized():
            nc.finalize()
        _cache["nc"] = nc
    nc = _cache["nc"]

    # host-side O(N*D) prologue
    yy = np.sum(y.astype(np.float64) * y.astype(np.float64), axis=1)
    b16 = yy.astype(np.float32).astype(ml_dtypes.bfloat16)
    z2 = 2.0 * np.sum(x.astype(np.float64) * y.astype(np.float64), axis=1)
    a = (yy - z2 + MARGIN).astype(np.float32)

    if USE_FP8:
        # [64, 2, cols] with contraction d = k2*64 + p, flattened to [64, 2*cols]
        xe = (2.0 * x).T.astype(ml_dtypes.float8_e4m3fn)  # [128, SH]
        ye = y.T.astype(ml_dtypes.float8_e4m3fn)          # [128, N]
        xtT = np.ascontiguousarray(
            xe.reshape(2, 64, N).transpose(1, 0, 2).reshape(64, 2 * N)
        )
        ytT = np.ascontiguousarray(
            ye.reshape(2, 64, N).transpose(1, 0, 2).reshape(64, 2 * N)
        )
    else:
        ytT = np.ascontiguousarray(y.T).astype(ml_dtypes.bfloat16)
        xtT = np.ascontiguousarray((2.0 * x).T).astype(ml_dtypes.bfloat16)

    bv = b16.reshape(1, N)
    in_maps = []
    for c in range(NCORES):
        sl = slice(c * SH, (c + 1) * SH)
        if USE_FP8:
            xc = xtT.reshape(64, 2, SH * NCORES)[:, :, sl].reshape(64, 2 * SH)
        else:
            xc = xtT[:, sl]
        in_maps.append({
            "xt": np.ascontiguousarray(xc),
            "yt": ytT,
            "bv": bv,
            "av": np.ascontiguousarray(a[sl].reshape(MT, 128).T),  # [128, MT]
        })

    _cache["in_maps"] = in_maps
    out = run_bass_kernel_spmd(nc, in_maps, list(range(NCORES)))
    results = out.results

    # host reduction (f64)
    total = 0.0
    for c in range(NCORES):
        total += np.asarray(results[c]["res"], dtype=np.float64).sum()
    bsum_blk = b16.astype(np.float64).reshape(NH, UNIT_W).sum(axis=1)
    # subtract Sum_b for every DVE (max-trick) unit
    for nh in range(NH):
        n_dve = sum(1 for m in range(MT) if not _is_act(m, nh))
        total -= NCORES * n_dve * 128.0 * bsum_blk[nh]
    total -= float(N) * float(np.float32(MARGIN))
    return np.float32(total / (float(N) * float(N)))


# revision 18
# speedup vs baseline: 1.5743x; 1.0009x over previous
"""HardTripletLoss (non-hardest branch) on 8 TRN2 NeuronCores — v2.3.

Math:  loss = mean_{i!=j} relu(d_pos[i] - pdist[i,j] + margin)
  pdist[i,j] = ||x_i||^2 + ||y_j||^2 - 2 x_i.y_j ,  d_pos = diag(pdist)
  =>  term(i,j) = relu(G[i,j] + a[i] - b[j])  with  G = 2 x y^T,
      a[i] = ||y_i||^2 - 2 x_i.y_i + margin,  b[j] = bf16(||y_j||^2).
Diagonal evaluates to ~relu(margin) = margin; host subtracts N*margin.

Host-side prep (O(N*D)): transpose + fp8/bf16-cast the matmul operands,
compute a and b.  Device does the O(N^2*D) matmul and the O(N^2)
relu/reduce epilogue.

G matmuls run in fp8 e4m3 DoubleRow mode (2 K-subtiles of 64 on 64
partitions -> 0.5 cycles/row).  Epilogue alternates DVE (max-trick) and
ACT (PE folds -b via K=1 bf16 matmuls, then Relu w/ bias=a) per m-tile.
Row-partial sums land in res [128, NU] per core; host reduces in f64.
"""

import sys

if "/opt/trn_rl_repo" not in sys.path:
    sys.path.insert(0, "/opt/trn_rl_repo")

import numpy as np

N, D = 8192, 128
NCORES = 8
SH = N // NCORES          # 1024 x-rows per core
MT = SH // 128            # 8 m-tiles
MARGIN = 0.2

UNIT_W = 1024             # epilogue tile width (psum: UNIT_W/512 banks)
NH = N // UNIT_W          # column blocks per row of m-tiles
NU = MT * NH              # total units per core
PS_BUFS = 8 * 512 // UNIT_W  # use all 8 psum banks
USE_FP8 = False


def _is_act(m, nh):
    # engine assignment: ACT (fold path) vs DVE (max-trick).
    # ACT units cost extra PE rows (the -b fold); ~12/64 balances
    # PE ~= (128 + 2a)*0.43us against DVE ~= (64-a)*1.28us.  The last
    # block leans ACT so the engines finish together (ACT/PE idle at
    # the end while DVE drains).
    if nh == 7:
        return m in (1, 4, 6)
    if nh % 2 == 0:
        return m in (2, 5)
    return m == 4


_cache = {}


def _build():
    import concourse.mybir as mybir
    from concourse import bacc
    from concourse.tile import TileContext

    f32 = mybir.dt.float32
    bf16 = mybir.dt.bfloat16
    fp8 = mybir.dt.float8e4
    mmdt = fp8 if USE_FP8 else bf16
    Alu = mybir.AluOpType
    Act = mybir.ActivationFunctionType
    DR = mybir.MatmulPerfMode.DoubleRow

    nc = bacc.Bacc()
    # host-pre-transposed operands: xt = (2x)^T, yt = y^T
    # fp8 DoubleRow layout: [64, 2, cols], contraction d = k2*64 + p
    if USE_FP8:
        xt = nc.declare_dram_parameter("xt", [64, 2 * SH], mmdt, isOutput=False)
        yt = nc.declare_dram_parameter("yt", [64, 2 * N], mmdt, isOutput=False)
    else:
        xt = nc.declare_dram_parameter("xt", [128, SH], mmdt, isOutput=False)
        yt = nc.declare_dram_parameter("yt", [128, N], mmdt, isOutput=False)
    bv = nc.declare_dram_parameter("bv", [1, N], bf16, isOutput=False)    # b row
    av = nc.declare_dram_parameter("av", [128, MT], f32, isOutput=False)  # a cols
    out_res = nc.declare_dram_parameter("res", [128, NU], f32, isOutput=True)

    with TileContext(nc) as tc:
        with (
            tc.tile_pool(name="big", bufs=1) as big,
            tc.tile_pool(name="epd", bufs=3) as epd,
            tc.tile_pool(name="epa", bufs=3) as epa,
            tc.tile_pool(name="ps", bufs=PS_BUFS, space="PSUM") as ps,
        ):
            if USE_FP8:
                yT = big.tile([64, 2, N], mmdt)      # y^T  [p, k2, j]
                xT = big.tile([64, 2, SH], mmdt)     # (2x)^T [p, k2, i]
            else:
                yT = big.tile([128, N], mmdt)
                xT = big.tile([128, SH], mmdt)
            bb = big.tile([128, N], bf16)            # b broadcast to all partitions
            brow = big.tile([1, N], bf16)            # b as a single row
            acol = big.tile([128, MT], f32)          # a per m-tile column
            negones = big.tile([1, 128], bf16)
            res = big.tile([128, NU], f32)

            nc.vector.memset(negones[:], -1.0)

            # ---- loads: yt chunks on sync ring; the rest on scalar ring ----
            # small leading chunks so the first matmuls / first DVE unit
            # start as early as possible.
            def _bcast(c0, c1):
                return bv[0:1, c0:c1].broadcast_to([128, c1 - c0])

            # sync ring: yt chunks interleaved with early bb chunks
            nc.sync.dma_start(yT[:, 0:1024], yt[:, 0:1024])
            nc.sync.dma_start(bb[:, 0:2048], _bcast(0, 2048))
            nc.sync.dma_start(yT[:, 1024:4096], yt[:, 1024:4096])
            nc.sync.dma_start(bb[:, 2048:4096], _bcast(2048, 4096))
            nc.sync.dma_start(yT[:, 4096:N], yt[:, 4096:N])
            # scalar ring: x operand + small vectors + late bb half
            nc.scalar.dma_start(xT[:], xt[:, :])
            nc.scalar.dma_start(acol[:], av[:, :])
            nc.scalar.dma_start(brow[:], bv[:, :])
            nc.scalar.dma_start(bb[:, 4096:N], _bcast(4096, N))

            # ---- main: G tiles + fused epilogue ----
            HW = UNIT_W // 512  # matmuls per unit
            for nh in range(NH):
                # interleave ACT units among DVE units.  Block 0 leads with
                # a DVE unit (bb[0:2048] lands before brow-dependent folds
                # finish), later blocks lead ACT.
                act_ms = [m for m in range(MT) if _is_act(m, nh)]
                dve_ms = [m for m in range(MT) if not _is_act(m, nh)]
                ms = []
                stride = max(1, len(dve_ms) // max(1, len(act_ms)))
                di = 0
                if nh == 0:
                    ms.extend(dve_ms[:2])
                    di = 2
                for am in act_ms:
                    ms.append(am)
                    ms.extend(dve_ms[di : di + stride])
                    di += stride
                ms.extend(dve_ms[di:])
                for m in ms:
                    col = m * NH + nh  # res column (m-major for host)
                    is_act = _is_act(m, nh)
                    pt = ps.tile([128, UNIT_W], f32, tag="g")
                    for h in range(HW):
                        c0 = nh * UNIT_W + h * 512
                        if USE_FP8:
                            nc.tensor.matmul(
                                pt[:, h * 512 : (h + 1) * 512],
                                lhsT=xT[:, :, m * 128 : (m + 1) * 128],
                                rhs=yT[:, :, c0 : c0 + 512],
                                start=True, stop=not is_act,
                                perf_mode=DR,
                            )
                        else:
                            nc.tensor.matmul(
                                pt[:, h * 512 : (h + 1) * 512],
                                lhsT=xT[:, m * 128 : (m + 1) * 128],
                                rhs=yT[:, c0 : c0 + 512],
                                start=True, stop=not is_act,
                            )
                    if is_act:
                        for h in range(HW):
                            c0 = nh * UNIT_W + h * 512
                            nc.tensor.matmul(
                                pt[:, h * 512 : (h + 1) * 512],
                                lhsT=negones[:],
                                rhs=brow[0:1, c0 : c0 + 512],
                                start=False, stop=True,
                            )
                        scr = epa.tile([128, UNIT_W], bf16, tag="ep_act")
                        nc.scalar.activation(
                            scr[:], pt[:], Act.Relu,
                            bias=acol[:, m : m + 1],
                            accum_out=res[:, col : col + 1],
                        )
                    else:
                        nc.vector.scalar_tensor_tensor(
                            out=pt[:], in0=pt[:], scalar=acol[:, m : m + 1],
                            in1=bb[:, nh * UNIT_W : (nh + 1) * UNIT_W],
                            op0=Alu.add, op1=Alu.max,
                            accum_out=res[:, col : col + 1],
                        )

            nc.scalar.dma_start(out_res[:], res[:])

    return nc


def kernel(x: np.ndarray, y: np.ndarray) -> np.ndarray:
    from concourse.bass_utils import run_bass_kernel_spmd
    import ml_dtypes

    x = np.ascontiguousarray(x, dtype=np.float32)
    y = np.ascontiguousarray(y, dtype=np.float32)

    if "nc" not in _cache:
        nc = _build()
        if not nc.is_finalized():
            nc.finalize()
        _cache["nc"] = nc
    nc = _cache["nc"]

    # host-side O(N*D) prologue
    yy = np.sum(y.astype(np.float64) * y.astype(np.float64), axis=1)
    b16 = yy.astype(np.float32).astype(ml_dtypes.bfloat16)
    z2 = 2.0 * np.sum(x.astype(np.float64) * y.astype(np.float64), axis=1)
    a = (yy - z2 + MARGIN).astype(np.float32)

    if USE_FP8:
        # [64, 2, cols] with contraction d = k2*64 + p, flattened to [64, 2*cols]
        xe = (2.0 * x).T.astype(ml_dtypes.float8_e4m3fn)  # [128, SH]
        ye = y.T.astype(ml_dtypes.float8_e4m3fn)          # [128, N]
        xtT = np.ascontiguousarray(
            xe.reshape(2, 64, N).transpose(1, 0, 2).reshape(64, 2 * N)
        )
        ytT = np.ascontiguousarray(
            ye.reshape(2, 64, N).transpose(1, 0, 2).reshape(64, 2 * N)
        )
    else:
        ytT = np.ascontiguousarray(y.T).astype(ml_dtypes.bfloat16)
        xtT = np.ascontiguousarray((2.0 * x).T).astype(ml_dtypes.bfloat16)

    bv = b16.reshape(1, N)
    in_maps = []
    for c in range(NCORES):
        sl = slice(c * SH, (c + 1) * SH)
        if USE_FP8:
            xc = xtT.reshape(64, 2, SH * NCORES)[:, :, sl].reshape(64, 2 * SH)
        else:
            xc = xtT[:, sl]
        in_maps.append({
            "xt": np.ascontiguousarray(xc),
            "yt": ytT,
            "bv": bv,
            "av": np.ascontiguousarray(a[sl].reshape(MT, 128).T),  # [128, MT]
        })

    _cache["in_maps"] = in_maps
    out = run_bass_kernel_spmd(nc, in_maps, list(range(NCORES)))
    results = out.results

    # host reduction (f64)
    total = 0.0
    for c in range(NCORES):
        total += np.asarray(results[c]["res"], dtype=np.float64).sum()
    bsum_blk = b16.astype(np.float64).reshape(NH, UNIT_W).sum(axis=1)
    # subtract Sum_b for every DVE (max-trick) unit
    for nh in range(NH):
        n_dve = sum(1 for m in range(MT) if not _is_act(m, nh))
        total -= NCORES * n_dve * 128.0 * bsum_blk[nh]
    total -= float(N) * float(np.float32(MARGIN))
    return np.float32(total / (float(N) * float(N)))
